# revision 6
# baseline (speedup 1.0000x reference)
"""GCMC graph-conv kernel for 8 Trainium2 NeuronCores.

Sharding: data-parallel over the batch (dst) dimension — core k owns batch rows
[k*1024, (k+1)*1024). Edge lists are partitioned by destination row on the
host, so every segment_sum is core-local and no collectives are needed. The
full fp16 feature tables are replicated to every core's HBM and edge features
are fetched with indirect gather DMA (dma_gather, int16 indices over four
25000-row table quarters). Weighted segment-sum runs on the TensorEngine as
one-hot matmuls whose selector matrices are built by a single fused DVE
tensor_scalar (is_equal x weight) per 128-edge chunk.
"""

from contextlib import ExitStack

import numpy as np

import concourse.bass as bass
import concourse.tile as tile
from concourse import bacc, mybir
from concourse.library_config import mlp

# Problem constants (hardcoded per contract)
N_ROWS = 100000      # rows in each feature table
D = 256              # feature dim
B = 8192             # global batch
BL = 1024            # batch rows per core
C = 5                # edge classes
E = 500000           # edges per class per side
DE = 256             # embedding dim
NB = 2               # decoder bases
NCORES = 8
QROWS = 25000        # table quarter size (int16-addressable)
NQ = 4
NG = 8               # dst blocks of 128 per core
CELL_CAP = 2048      # edges per (class, g, q) cell; overflow goes to spill
NQUEUES = 4          # SWDGE queues; gathers round-robin so transfers overlap
DMA_SCRATCH = 16384  # SWDGE descriptor-ring carveout bytes per partition
CELL_CH = CELL_CAP // 128        # 16 chunks per cell
SC_CH = NG * NQ * CELL_CH        # main chunks per (side, class) = 512
GATHERS_SC = NG * NQ * (CELL_CAP // 1024)   # 64 gathers of 1024 per (s,c)
BN_EPS = 1e-3

F16 = mybir.dt.float16
F32 = mybir.dt.float32
I16 = mybir.dt.int16
I32 = mybir.dt.int32

_CACHE = {}

import os as _os
_SKIP = set(_os.environ.get("K_SKIP", "").split(","))


def _build(reps: int = 1, dbg: bool = False):
    """Trace + compile the SPMD single-core program."""
    nc = bacc.Bacc("TRN2", target_bir_lowering=False, debug=False,
                   num_devices=NCORES, num_swdge_queues=NQUEUES,
                   dynamic_dma_scratch_size=DMA_SCRATCH)

    def din(name, shape, dt):
        return nc.dram_tensor(name, shape, dt, kind="ExternalInput").ap()

    tabs = {0: din("tab_item", [N_ROWS, D], F16),   # side 0 (user) gathers items
            1: din("tab_user", [N_ROWS, D], F16)}   # side 1 (item) gathers users
    idx_e = [din(f"idx_e{s}", [128, C * GATHERS_SC * 64], I16) for s in range(2)]
    dcol = [din(f"dcol{s}", [128, C * SC_CH], F32) for s in range(2)]
    wcol = [din(f"wcol{s}", [128, C * SC_CH], F32) for s in range(2)]
    isp = [din(f"isp{s}", [128, C], I32) for s in range(2)]
    dsp = [din(f"dsp{s}", [128, C], F32) for s in range(2)]
    wsp = [din(f"wsp{s}", [128, C], F32) for s in range(2)]
    bidx = [din(f"bidx{s}", [128, 8], I32) for s in range(2)]
    scale_f = [din(f"scale_f{s}", [128, 2], F32) for s in range(2)]
    bias_f = [din(f"bias_f{s}", [128, 2], F32) for s in range(2)]
    scale_h = [din(f"scale_h{s}", [128, C], F32) for s in range(2)]
    bias_h = [din(f"bias_h{s}", [128, C], F32) for s in range(2)]
    w_f = [din(f"w_f{s}", [128, 2, 2, 128], F16) for s in range(2)]
    w_c = [din(f"w_c{s}", [128, C, 2, 128], F16) for s in range(2)]
    w2_f = [din(f"w2_f{s}", [128, 2, 2, 128], F16) for s in range(2)]
    w2_h = [din(f"w2_h{s}", [128, C, 2, 128], F16) for s in range(2)]
    wdec = din("wdec", [128, NB, 2, 2, 128], F16)
    wcomb = din("wcomb", [128, NB, C], F32)
    iota = din("iota", [128, NG, 128], F16)
    ident = din("ident", [128, 128], F32)
    out = nc.dram_tensor("logitsT", [C, BL], F32, kind="ExternalOutput").ap()
    dbg_out = (nc.dram_tensor("dbg_agg", [NG * 128, D], F32, kind="ExternalOutput").ap()
               if dbg else None)

    with tile.TileContext(nc) as tc:
        nc.gpsimd.load_library(mlp)
        with ExitStack() as ctx:
            cpool = ctx.enter_context(tc.tile_pool(name="const", bufs=1))
            idxp = ctx.enter_context(tc.tile_pool(name="idx", bufs=2))
            wdp = ctx.enter_context(tc.tile_pool(name="wd", bufs=4))
            gp = ctx.enter_context(tc.tile_pool(name="gath", bufs=8))
            # deep S pool: DVE must build selectors far ahead of the gathers
            # so matmuls burst when a gather lands instead of JIT-chaining
            sp = ctx.enter_context(tc.tile_pool(name="sel", bufs=32))
            # PSUM: 8 banks total; every tile pads to a full bank, and a
            # start=True matmul clobbers other accumulations in its bank, so
            # concurrent accumulators must sit in distinct banks.
            aggp = ctx.enter_context(tc.tile_pool(name="aggps", bufs=3, space="PSUM"))
            tpsp = ctx.enter_context(tc.tile_pool(name="tpsps", bufs=2, space="PSUM"))
            dnsp = ctx.enter_context(tc.tile_pool(name="dnsps", bufs=2, space="PSUM"))
            sbp = ctx.enter_context(tc.tile_pool(name="sbt", bufs=3))
            perp = ctx.enter_context(tc.tile_pool(name="persist", bufs=1))

            # ---- persistent constants ----
            iot = cpool.tile([128, NG, 128], F16)
            nc.sync.dma_start(iot[:], iota[:])
            idn = cpool.tile([128, 128], F32)
            nc.sync.dma_start(idn[:], ident[:])
            ones = cpool.tile([128, 1], F32)
            nc.vector.memset(ones[:], 1.0)
            wcomb_sb = cpool.tile([128, NB, C], F32)
            nc.sync.dma_start(wcomb_sb[:], wcomb[:])
            wdec_sb = cpool.tile([128, NB, 2, 2, 128], F16)
            nc.sync.dma_start(wdec_sb[:], wdec[:])
            wf_sb, wc_sb, w2f_sb, w2h_sb = [], [], [], []
            sclf_sb, biaf_sb, sclh_sb, biah_sb = [], [], [], []
            isp_sb, dsp_sb, wsp_sb, bidx_sb = [], [], [], []
            for s in range(2):
                t = cpool.tile([128, 2, 2, 128], F16, tag=f"wf{s}")
                nc.sync.dma_start(t[:], w_f[s][:]); wf_sb.append(t)
                t = cpool.tile([128, C, 2, 128], F16, tag=f"wc{s}")
                nc.sync.dma_start(t[:], w_c[s][:]); wc_sb.append(t)
                t = cpool.tile([128, 2, 2, 128], F16, tag=f"w2f{s}")
                nc.sync.dma_start(t[:], w2_f[s][:]); w2f_sb.append(t)
                t = cpool.tile([128, C, 2, 128], F16, tag=f"w2h{s}")
                nc.sync.dma_start(t[:], w2_h[s][:]); w2h_sb.append(t)
                for src_ap, lst, tg in ((scale_f[s], sclf_sb, "sf"), (bias_f[s], biaf_sb, "bf"),
                                        (scale_h[s], sclh_sb, "sh"), (bias_h[s], biah_sb, "bh")):
                    t = cpool.tile(list(src_ap.shape), F32, tag=f"{tg}{s}")
                    nc.sync.dma_start(t[:], src_ap[:]); lst.append(t)
                t = cpool.tile([128, C], I32, tag=f"isp{s}")
                nc.sync.dma_start(t[:], isp[s][:]); isp_sb.append(t)
                t = cpool.tile([128, C], F32, tag=f"dsp{s}")
                nc.sync.dma_start(t[:], dsp[s][:]); dsp_sb.append(t)
                t = cpool.tile([128, C], F32, tag=f"wsp{s}")
                nc.sync.dma_start(t[:], wsp[s][:]); wsp_sb.append(t)
                t = cpool.tile([128, 8], I32, tag=f"bidx{s}")
                nc.sync.dma_start(t[:], bidx[s][:]); bidx_sb.append(t)

            # persistent activations
            hT = [[perp.tile([128, BL], F16, tag=f"hT{s}_{c}", name=f"hT{s}_{c}") for c in range(C)]
                  for s in range(2)]
            fT = [perp.tile([128, 2, BL], F16, tag=f"fT{s}", name=f"fT{s}") for s in range(2)]
            uT = [perp.tile([128, 2, BL], F16, tag=f"uT{s}", name=f"uT{s}") for s in range(2)]
            qT = [perp.tile([128, 2, BL], F16, tag=f"qT{k}", name=f"qT{k}") for k in range(NB)]
            featT = perp.tile([128, 2, BL], F16, tag="featT")

            def body():
                gq = [0]  # round-robin SWDGE queue cursor

                # ============ graph conv: gather + segment sum + project ====
                for s in range(2):
                    tab = tabs[s]
                    for c in range(C):
                        idx_t = idxp.tile([128, GATHERS_SC * 64], I16, tag="idxt")
                        nc.sync.dma_start(
                            idx_t[:], idx_e[s][:, c * GATHERS_SC * 64:(c + 1) * GATHERS_SC * 64])
                        dcol_t = wdp.tile([128, SC_CH], F32, tag="dcolt")
                        nc.sync.dma_start(dcol_t[:], dcol[s][:, c * SC_CH:(c + 1) * SC_CH])
                        wcol_t = wdp.tile([128, SC_CH], F32, tag="wcolt")
                        nc.sync.dma_start(wcol_t[:], wcol[s][:, c * SC_CH:(c + 1) * SC_CH])

                        # spill rows for this (s,c): one 128-edge chunk
                        spg = gp.tile([128, D], F16, tag="spg")
                        nc.gpsimd.indirect_dma_start(
                            out=spg[:], out_offset=None, in_=tab[:],
                            in_offset=bass.IndirectOffsetOnAxis(
                                ap=isp_sb[s][:, c:c + 1], axis=0))
                        gi = 0
                        for g in range(NG):
                            agg = aggp.tile([128, D], F32, tag="agg")
                            # spill contribution first so the accumulation
                            # group is contiguous per g
                            S = sp.tile([128, 128], F16, tag="S")
                            nc.vector.tensor_scalar(
                                out=S[:], in0=iot[:, g, :],
                                scalar1=dsp_sb[s][:, c:c + 1],
                                scalar2=wsp_sb[s][:, c:c + 1],
                                op0=mybir.AluOpType.is_equal,
                                op1=mybir.AluOpType.mult)
                            nc.tensor.matmul(agg[:], lhsT=S[:], rhs=spg[:],
                                             start=True, stop=False,
                                             skip_group_check=True)
                            for q in range(NQ):
                                q0 = q * QROWS
                                for half in range(CELL_CAP // 1024):
                                    gt = gp.tile([128, 8, D], F16, tag="gt")
                                    if "gather" not in _SKIP:
                                        nc.gpsimd.dma_gather(
                                            gt[:], tab[q0:q0 + QROWS, :],
                                            idx_t[:, gi * 64:(gi + 1) * 64],
                                            1024, 1024, D,
                                            queue_num=gq[0] % NQUEUES)
                                        gq[0] += 1
                                    for j in range(8):
                                        ch = gi * 8 + j
                                        S = sp.tile([128, 128], F16, tag="S")
                                        if "dve" not in _SKIP:
                                            nc.vector.tensor_scalar(
                                                out=S[:], in0=iot[:, 0, :],
                                                scalar1=dcol_t[:, ch:ch + 1],
                                                scalar2=wcol_t[:, ch:ch + 1],
                                                op0=mybir.AluOpType.is_equal,
                                                op1=mybir.AluOpType.mult)
                                        if "mm" not in _SKIP:
                                            nc.tensor.matmul(
                                                agg[:], lhsT=S[:], rhs=gt[:, j, :],
                                                start=False,
                                                stop=(q == NQ - 1 and half == 1 and j == 7),
                                                skip_group_check=True)
                                    gi += 1
                            # drain: transpose, project, bn+relu -> hT
                            agg_sb = sbp.tile([128, D], F32, tag="aggsb")
                            nc.scalar.copy(agg_sb[:], agg[:])
                            if dbg and s == 0 and c == 0:
                                nc.sync.dma_start(dbg_out[g * 128:(g + 1) * 128, :], agg_sb[:])
                            aggT = sbp.tile([128, D], F16, tag="aggT")
                            for hh in range(2):
                                tps = tpsp.tile([128, 128], F32, tag="tps")
                                nc.tensor.transpose(
                                    tps[:], agg_sb[:, hh * 128:(hh + 1) * 128], idn[:])
                                nc.vector.tensor_copy(
                                    aggT[:, hh * 128:(hh + 1) * 128], tps[:])
                            hps = dnsp.tile([128, 128], F32, tag="dense")
                            for hh in range(2):
                                nc.tensor.matmul(
                                    hps[:], lhsT=wc_sb[s][:, c, hh, :],
                                    rhs=aggT[:, hh * 128:(hh + 1) * 128],
                                    start=(hh == 0), stop=(hh == 1),
                                    skip_group_check=True)
                            nc.scalar.activation(
                                hT[s][c][:, g * 128:(g + 1) * 128],
                                hps[:],
                                mybir.ActivationFunctionType.Relu,
                                bias=biah_sb[s][:, c:c + 1],
                                scale=sclh_sb[s][:, c:c + 1])

                # ============ dense f path ============
                ftab = {0: tabs[1], 1: tabs[0]}  # f_user reads user table
                for s in range(2):
                    for j in range(8):
                        bg = gp.tile([128, D], F16, tag="bg")
                        nc.gpsimd.indirect_dma_start(
                            out=bg[:], out_offset=None, in_=ftab[s][:],
                            in_offset=bass.IndirectOffsetOnAxis(
                                ap=bidx_sb[s][:, j:j + 1], axis=0))
                        bgf = sbp.tile([128, D], F32, tag="aggsb")
                        nc.scalar.copy(bgf[:], bg[:])
                        for hh in range(2):
                            tps = tpsp.tile([128, 128], F32, tag="tps")
                            nc.tensor.transpose(
                                tps[:], bgf[:, hh * 128:(hh + 1) * 128], idn[:])
                            nc.vector.tensor_copy(
                                featT[:, hh, j * 128:(j + 1) * 128], tps[:])
                    for mh in range(2):
                        for n in range(2):
                            fps = dnsp.tile([128, 512], F32, tag="dense")
                            for kh in range(2):
                                nc.tensor.matmul(
                                    fps[:], lhsT=wf_sb[s][:, kh, mh, :],
                                    rhs=featT[:, kh, n * 512:(n + 1) * 512],
                                    start=(kh == 0), stop=(kh == 1))
                            nc.scalar.activation(
                                fT[s][:, mh, n * 512:(n + 1) * 512], fps[:],
                                mybir.ActivationFunctionType.Relu,
                                bias=biaf_sb[s][:, mh:mh + 1],
                                scale=sclf_sb[s][:, mh:mh + 1])

                # ============ embeddings u = relu(f@W2f + h@W2h) ============
                for s in range(2):
                    for mh in range(2):
                        for n in range(2):
                            ups = dnsp.tile([128, 512], F32, tag="dense")
                            for kh in range(2):
                                nc.tensor.matmul(
                                    ups[:], lhsT=w2f_sb[s][:, kh, mh, :],
                                    rhs=fT[s][:, kh, n * 512:(n + 1) * 512],
                                    start=(kh == 0), stop=False,
                                    skip_group_check=True)
                            for c in range(C):
                                nc.tensor.matmul(
                                    ups[:], lhsT=w2h_sb[s][:, c, mh, :],
                                    rhs=hT[s][c][:, n * 512:(n + 1) * 512],
                                    start=False, stop=(c == C - 1),
                                    skip_group_check=True)
                            nc.scalar.activation(
                                uT[s][:, mh, n * 512:(n + 1) * 512], ups[:],
                                mybir.ActivationFunctionType.Relu)

                # ============ bilinear decoder ============
                for k in range(NB):
                    for jh in range(2):
                        for n in range(2):
                            qps = dnsp.tile([128, 512], F32, tag="dense")
                            for ih in range(2):
                                nc.tensor.matmul(
                                    qps[:], lhsT=wdec_sb[:, k, ih, jh, :],
                                    rhs=uT[0][:, ih, n * 512:(n + 1) * 512],
                                    start=(ih == 0), stop=(ih == 1))
                            nc.scalar.copy(qT[k][:, jh, n * 512:(n + 1) * 512], qps[:])
                lg = sbp.tile([C, BL], F32, tag="lg")
                for n in range(2):
                    lps = dnsp.tile([C, 512], F32, tag="dense")
                    for k in range(NB):
                        for jh in range(2):
                            r = sbp.tile([128, 512], F32, tag="r")
                            nc.vector.tensor_tensor(
                                out=r[:], in0=qT[k][:, jh, n * 512:(n + 1) * 512],
                                in1=uT[1][:, jh, n * 512:(n + 1) * 512],
                                op=mybir.AluOpType.mult)
                            nc.tensor.matmul(
                                lps[:], lhsT=wcomb_sb[:, k, :], rhs=r[:],
                                start=(k == 0 and jh == 0),
                                stop=(k == NB - 1 and jh == 1),
                                skip_group_check=True)
                    nc.scalar.copy(lg[:, n * 512:(n + 1) * 512], lps[:])
                nc.sync.dma_start(out[:], lg[:])

            if reps == 1:
                body()
            else:
                with tc.For_i(0, reps, 1):
                    body()

    nc.compile()
    return nc


# ---------------------------------------------------------------------------
# Host-side sharding / preprocessing
# ---------------------------------------------------------------------------

def _wrap_idx16(stream):
    """int16 stream (multiple of 1024) -> [128, len/16] dma_gather layout."""
    nb = stream.shape[0] // 1024
    a = stream.reshape(nb, 64, 16).transpose(0, 2, 1)      # [nb, 16, 64]
    a = np.tile(a, (1, 8, 1))                              # [nb, 128, 64]
    return np.ascontiguousarray(a.transpose(1, 0, 2).reshape(128, nb * 64))


def _prep_side(src, dst, w, core):
    """Per-core per-side edge prep. src/dst/w are [C, E] full arrays."""
    lo, hi = core * BL, (core + 1) * BL
    idx_cols, dcols, wcols = [], [], []
    ispv = np.zeros((128, C), np.int32)
    dspv = np.zeros((128, C), np.float32)
    wspv = np.zeros((128, C), np.float32)
    for c in range(C):
        m = (dst[c] >= lo) & (dst[c] < hi)
        sc, dl_full, wc_ = src[c][m], dst[c][m] - lo, w[c][m]
        g = dl_full >> 7
        q = sc // QROWS
        key = g * NQ + q
        order = np.argsort(key, kind="stable")
        sc, dl_full, wc_, key = sc[order], dl_full[order], wc_[order], key[order]
        bounds = np.searchsorted(key, np.arange(NG * NQ + 1))
        stream_i = np.zeros(NG * NQ * CELL_CAP, np.int16)
        stream_d = np.zeros(NG * NQ * CELL_CAP, np.float32)
        stream_w = np.zeros(NG * NQ * CELL_CAP, np.float32)
        sp_i, sp_d, sp_w = [], [], []
        for cell in range(NG * NQ):
            a, b = bounds[cell], bounds[cell + 1]
            n = b - a
            take = min(n, CELL_CAP)
            base = cell * CELL_CAP
            qq = cell % NQ
            stream_i[base:base + take] = (sc[a:a + take] - qq * QROWS).astype(np.int16)
            stream_d[base:base + take] = (dl_full[a:a + take] & 127).astype(np.float32)
            stream_w[base:base + take] = wc_[a:a + take]
            if n > take:
                sp_i.append(sc[a + take:b])
                sp_d.append(dl_full[a + take:b])
                sp_w.append(wc_[a + take:b])
        nsp = sum(x.size for x in sp_i)
        if nsp > 128:
            raise RuntimeError(f"spill overflow: {nsp} > 128")
        if nsp:
            ispv[:nsp, c] = np.concatenate(sp_i)
            dspv[:nsp, c] = np.concatenate(sp_d).astype(np.float32)
            wspv[:nsp, c] = np.concatenate(sp_w)
        idx_cols.append(_wrap_idx16(stream_i))
        dcols.append(stream_d.reshape(-1, 128).T)
        wcols.append(stream_w.reshape(-1, 128).T)
    return (np.concatenate(idx_cols, axis=1),
            np.ascontiguousarray(np.concatenate(dcols, axis=1), dtype=np.float32),
            np.ascontiguousarray(np.concatenate(wcols, axis=1), dtype=np.float32),
            ispv, dspv, wspv)


def _bn_fold(bn, lin_bias=None):
    gamma, beta, mean, var = (bn[0].astype(np.float64), bn[1].astype(np.float64),
                              bn[2].astype(np.float64), bn[3].astype(np.float64))
    a = gamma / np.sqrt(var + BN_EPS)
    b = beta - mean * a
    if lin_bias is not None:
        b = b + a * lin_bias.astype(np.float64)
    return a.astype(np.float32), b.astype(np.float32)


def _as_cols(v, ncol):
    return np.ascontiguousarray(v.reshape(ncol, 128).T, dtype=np.float32)


def _make_inputs(inp, nc_unused=None):
    item_h = np.asarray(inp["item_feat"]).astype(np.float16)
    user_h = np.asarray(inp["user_feat"]).astype(np.float16)

    def wf_layout(W):   # [256,256] -> [128, kh, mh, 128]
        return np.ascontiguousarray(
            np.asarray(W).reshape(2, 128, 2, 128).transpose(1, 0, 2, 3)).astype(np.float16)

    def wc_layout(W):   # [C,256,128] -> [128, C, kh, 128]
        return np.ascontiguousarray(
            np.asarray(W).reshape(C, 2, 128, 128).transpose(2, 0, 1, 3)).astype(np.float16)

    def w2h_layout(W):  # [640,256] -> [128, C, mh, 128]
        return np.ascontiguousarray(
            np.asarray(W).reshape(C, 128, 2, 128).transpose(1, 0, 2, 3)).astype(np.float16)

    wdec_dev = np.ascontiguousarray(
        np.asarray(inp["Wdec"]).reshape(NB, 2, 128, 2, 128).transpose(2, 0, 1, 3, 4)).astype(np.float16)
    wcomb_dev = np.ascontiguousarray(
        np.tile(np.asarray(inp["Wcomb"]).T[None, :, :], (128, 1, 1))).astype(np.float32)
    iota_dev = (np.arange(NG * 128, dtype=np.float32).reshape(NG, 128)[None, :, :]
                + np.zeros((128, 1, 1), np.float32)).astype(np.float16)
    ident_dev = np.eye(128, dtype=np.float32)

    af_u, bf_u = _bn_fold(np.asarray(inp["bn_fu"]), np.asarray(inp["b_fu"]))
    af_i, bf_i = _bn_fold(np.asarray(inp["bn_fi"]), np.asarray(inp["b_fi"]))
    ah_u, bh_u = _bn_fold(np.asarray(inp["bn_hu"]))
    ah_i, bh_i = _bn_fold(np.asarray(inp["bn_hi"]))

    shared = {
        "tab_item": item_h, "tab_user": user_h,
        "w_f0": wf_layout(inp["W_fu"]), "w_f1": wf_layout(inp["W_fi"]),
        "w_c0": wc_layout(inp["W_uc"]), "w_c1": wc_layout(inp["W_ic"]),
        "w2_f0": wf_layout(inp["W2_fu"]), "w2_f1": wf_layout(inp["W2_fi"]),
        "w2_h0": w2h_layout(inp["W2_hu"]), "w2_h1": w2h_layout(inp["W2_hi"]),
        "wdec": wdec_dev, "wcomb": wcomb_dev, "iota": iota_dev, "ident": ident_dev,
        "scale_f0": _as_cols(af_u, 2), "bias_f0": _as_cols(bf_u, 2),
        "scale_f1": _as_cols(af_i, 2), "bias_f1": _as_cols(bf_i, 2),
        "scale_h0": _as_cols(ah_u, C), "bias_h0": _as_cols(bh_u, C),
        "scale_h1": _as_cols(ah_i, C), "bias_h1": _as_cols(bh_i, C),
    }

    in_maps = []
    ues, ued, uew = (np.asarray(inp["user_edge_src"]), np.asarray(inp["user_edge_dst"]),
                     np.asarray(inp["user_edge_w"]))
    ies, ied, iew = (np.asarray(inp["item_edge_src"]), np.asarray(inp["item_edge_dst"]),
                     np.asarray(inp["item_edge_w"]))
    uidx = np.asarray(inp["user_idx"])
    iidx = np.asarray(inp["item_idx"])
    for k in range(NCORES):
        m = dict(shared)
        i0, d0, w0, is0, ds0, ws0 = _prep_side(ues, ued, uew, k)
        i1, d1, w1, is1, ds1, ws1 = _prep_side(ies, ied, iew, k)
        m.update({"idx_e0": i0, "dcol0": d0, "wcol0": w0,
                  "isp0": is0, "dsp0": ds0, "wsp0": ws0,
                  "idx_e1": i1, "dcol1": d1, "wcol1": w1,
                  "isp1": is1, "dsp1": ds1, "wsp1": ws1,
                  "bidx0": np.ascontiguousarray(
                      uidx[k * BL:(k + 1) * BL].reshape(8, 128).T).astype(np.int32),
                  "bidx1": np.ascontiguousarray(
                      iidx[k * BL:(k + 1) * BL].reshape(8, 128).T).astype(np.int32)})
        in_maps.append(m)
    return in_maps


def kernel(**inp):
    nc = _CACHE.get("nc")
    if nc is None:
        nc = _build()
        _CACHE["nc"] = nc
    in_maps = _make_inputs(inp)
    from concourse.bass_utils import run_bass_kernel_spmd
    res = run_bass_kernel_spmd(nc, in_maps, core_ids=list(range(NCORES)))
    outp = np.concatenate([res.results[k]["logitsT"].T for k in range(NCORES)], axis=0)
    return outp.astype(np.float32)

